# revision 21
# baseline (speedup 1.0000x reference)
"""Trainium2 Bass kernel for nn_CVXPolicy_Quadcopter.

Computes, for each of B=500000 samples:
    p = MLP(concat([t, z]));  c = [(p6+p7+p8)/m, p9, p10, p11]
    ustar = -c * exp(-0.5 * W(||c||^2))   (W = Lambert W, via Newton)

Strategy (pure data parallel over 8 cores, feature-major MLP):
  - host: inp_aug [14, B_pad] = [t; z^T; ones] (ones row folds b1 into W1).
    W2 columns are pre-combined and NEGATED (W2cn) so the device computes
    -c directly; ||c||^2 is sign-invariant and ustar = (-c) * exp(-w/2).
  - device per core (65536 samples = 128 tiles of 512):
      mm1: h_pre[100, 512] = W1a^T @ inp_tile  (PE, K=14)
      tanh: ACT over [100, 2048] (4 tiles / PSUM-bank group)
      mm2: c[4, 512] per tile, packed 3 tiles/PSUM bank at offsets {0,32,64}
      bias-copy: PSUM -> SBUF with per-partition bias (DVE), then per-tile
      DMA folds [4, 512] -> c_all[tile, 2048] (component-major planes)
      dense phase on [128, 512]: x = ||c||^2, Lambert W via 4 Newton iters
      (init = exponent-bit log1p approx; exp on ACT, rest on DVE),
      ustar planes = c_all * exp(-w/2), one DMA out to u_t [4, 65536]
  - host: gather per-core u_t, transpose to [B, 4].
"""

import numpy as np

_B = 500000
_NCORES = 8
_BLOC = 65536            # per-core padded batch: 128 tiles x 512
_BPAD = _BLOC * _NCORES
_NT = 512                # samples per tile
_NTILES = _BLOC // _NT   # 128
_MASS = 0.5
# w0 = C * (int_bits(1 + x) - B): tuned so that 4 Newton iters reach fp32 eps
_LOG_B = 1064866805.0
_LOG_C = 6.197218803882235e-08
_NEWTON_ITERS = 4

_CACHE = {}


def _build_nc():
    import concourse.bacc as bacc
    import concourse.tile as tile
    from concourse import mybir

    f32 = mybir.dt.float32
    f16 = mybir.dt.float16
    i32 = mybir.dt.int32
    AF = mybir.ActivationFunctionType
    ALU = mybir.AluOpType

    nc = bacc.Bacc("TRN2", target_bir_lowering=False, debug=False,
                   num_devices=_NCORES)

    inp = nc.dram_tensor("inp", [14, _BLOC], f16, kind="ExternalInput")
    w1a = nc.dram_tensor("w1a", [14, 100], f16, kind="ExternalInput")
    w2cn = nc.dram_tensor("w2cn", [100, 32], f16, kind="ExternalInput")
    b2s = nc.dram_tensor("b2s", [100, 1], f32, kind="ExternalInput")
    out = nc.dram_tensor("out", [4, _BLOC], f32, kind="ExternalOutput")

    with tile.TileContext(nc) as tc:
        with (
            tc.tile_pool(name="consts", bufs=1) as consts,
            tc.tile_pool(name="inpp", bufs=3) as inp_pool,
            tc.tile_pool(name="hs", bufs=3) as h_pool,
            tc.tile_pool(name="csb", bufs=3) as csb_pool,
            tc.tile_pool(name="big", bufs=1) as big_pool,
            tc.tile_pool(name="nt", bufs=2) as nt_pool,
            tc.tile_pool(name="hp", bufs=2, space="PSUM") as hp_pool,
            tc.tile_pool(name="cps", bufs=2, space="PSUM") as c_pool,
            tc.tile_pool(name="dram", bufs=1, space="DRAM") as dram_pool,
        ):
            w1a_sb = consts.tile([14, 100], f16, tag="w1a")
            nc.sync.dma_start(w1a_sb, w1a[:])
            w2c_sb = consts.tile([100, 32], f16, tag="w2c")
            nc.sync.dma_start(w2c_sb, w2cn[:])
            b2_sb = consts.tile([100, 1], f32, tag="b2s")
            nc.sync.dma_start(b2_sb, b2s[:])

            c_all = big_pool.tile([128, 2048], f32, tag="c_all")
            u_all = big_pool.tile([128, 2048], f32, tag="u_all")
            half = consts.tile([128, 1], f32, tag="half")
            nc.vector.memset(half, 0.5)
            # DRAM scratch for the c densify round-trip: triad g's biased
            # c block [96, 512] is dumped verbatim; the readback gathers the
            # real rows {32k+i} of every tile with DRAM-side addressing
            scratch = dram_pool.tile([43, 96, 512], f32, tag="scratch")

            # dense-phase tiles (written in row halves; A half mid-loop)
            sq = nt_pool.tile([128, 2048], f32, tag="sq", bufs=1)
            s2 = nt_pool.tile([128, 1024], f32, tag="s2", bufs=1)
            x = nt_pool.tile([128, _NT], f32, tag="x", bufs=1)
            xe = nt_pool.tile([128, _NT], f32, tag="xe", bufs=1)
            y = nt_pool.tile([128, _NT], f32, tag="y", bufs=1)
            fi = nt_pool.tile([128, _NT], f32, tag="fi", bufs=1)
            v = nt_pool.tile([128, _NT], f32, tag="v")

            def dense_half(r0, r1):
                nc.vector.tensor_mul(sq[r0:r1], c_all[r0:r1], c_all[r0:r1])
                nc.vector.tensor_add(s2[r0:r1], sq[r0:r1, 0:1024],
                                     sq[r0:r1, 1024:2048])
                nc.vector.tensor_add(x[r0:r1], s2[r0:r1, 0:512],
                                     s2[r0:r1, 512:1024])
                nc.vector.tensor_scalar_add(y[r0:r1], x[r0:r1], 1.0)
                nc.vector.tensor_scalar_mul(xe[r0:r1], x[r0:r1],
                                            float(np.e))
                nc.vector.tensor_copy(fi[r0:r1], y[r0:r1].bitcast(i32))
                # v0 = C*(bits - B) + 1 = bits*C - (B*C - 1)
                nc.vector.tensor_scalar(
                    v[r0:r1], fi[r0:r1], _LOG_C, _LOG_B * _LOG_C - 1.0,
                    op0=ALU.mult, op1=ALU.subtract)

            NG = (_NTILES + 2) // 3   # 43 triads (last has 2 tiles)
            it_ring = {}
            for g in range(NG):
                ntr = min(3, _NTILES - 3 * g)
                t0 = 3 * g
                # issue the input DMA for any octet starting in this triad
                for tt in range(t0, t0 + ntr):
                    if tt % 8 == 0:
                        o = tt // 8
                        it_new = inp_pool.tile([14, 4096], f16, tag="inp")
                        it_ring[o] = it_new
                        nc.sync.dma_start(
                            it_new, inp[:, 4096 * o:4096 * (o + 1)])
                hp_cur = hp_pool.tile([100, 512 * ntr], f32, tag="hp")
                for k in range(ntr):
                    tt = t0 + k
                    nc.tensor.matmul(
                        hp_cur[:, _NT * k:_NT * (k + 1)],
                        lhsT=w1a_sb[:],
                        rhs=it_ring[tt // 8][:, _NT * (tt % 8):
                                             _NT * (tt % 8 + 1)],
                        start=True, stop=True,
                    )
                h_cur = h_pool.tile([100, 512 * ntr], f16, tag="h")
                nc.scalar.activation(h_cur, hp_cur, AF.Tanh)
                cps_cur = c_pool.tile([128, _NT], f32, tag="c")
                for k in range(ntr):
                    nc.tensor.matmul(
                        cps_cur[32 * k:32 * k + 32, :],
                        lhsT=w2c_sb[:],
                        rhs=h_cur[:, _NT * k:_NT * (k + 1)],
                        start=True, stop=True,
                    )
                nr = 32 * ntr
                cc = csb_pool.tile([128, _NT], f32, tag="csb")
                nc.vector.tensor_scalar_add(
                    cc[0:nr, :], cps_cur[0:nr, :], b2_sb[0:nr, :])
                # dump triad (incl. garbage rows) verbatim to DRAM scratch
                # on the otherwise-idle software DGE
                nc.gpsimd.dma_start(scratch[g, 0:nr, :], cc[0:nr, :])
                if g == 21:
                    # tiles 0..63 are all dumped; gather them now so the
                    # readback overlaps the remaining loop iterations
                    srcA = scratch[:].rearrange(
                        "g (k r) n -> (g k) r n", k=3)[0:64, 0:4, :]
                    nc.sync.dma_start(
                        c_all[0:64, :].rearrange("p (i n) -> p i n", i=4),
                        srcA)
                    dense_half(0, 64)

            srcB = scratch[:].rearrange(
                "g (k r) n -> (g k) r n", k=3)[64:128, 0:4, :]
            nc.sync.dma_start(
                c_all[64:128, :].rearrange("p (i n) -> p i n", i=4), srcB)

            dense_half(64, 128)
            for _ in range(_NEWTON_ITERS):
                f = nt_pool.tile([128, _NT], f32, tag="f")
                nc.scalar.activation(f, v, AF.Exp, scale=-1.0)
                rv = nt_pool.tile([128, _NT], f32, tag="rv")
                nc.vector.reciprocal(rv, v)
                p = nt_pool.tile([128, _NT], f32, tag="p")
                nc.vector.scalar_tensor_tensor(
                    p, v, 1.0, v, op0=ALU.subtract, op1=ALU.mult)
                tt_ = nt_pool.tile([128, _NT], f32, tag="tt")
                nc.vector.tensor_mul(tt_, xe, f)
                num = nt_pool.tile([128, _NT], f32, tag="num")
                nc.vector.scalar_tensor_tensor(
                    num, p, 1.0, tt_, op0=ALU.add, op1=ALU.add)
                vn = nt_pool.tile([128, _NT], f32, tag="v")
                nc.vector.tensor_mul(vn, num, rv)
                v = vn

            # ne = exp(-w/2) = exp(-(v-1)/2) = exp(-0.5*v + 0.5)
            ne = nt_pool.tile([128, _NT], f32, tag="ne", bufs=1)
            nc.scalar.activation(ne, v, AF.Exp, scale=-0.5, bias=half[:])

            ne2 = nt_pool.tile([128, 1024], f32, tag="ne2", bufs=1)
            nc.vector.tensor_copy(ne2[:, 0:512], ne)
            nc.vector.tensor_copy(ne2[:, 512:1024], ne)
            for jp in range(2):
                nc.vector.tensor_mul(
                    u_all[:, 1024 * jp:1024 * (jp + 1)],
                    c_all[:, 1024 * jp:1024 * (jp + 1)],
                    ne2,
                )
                dst = out[2 * jp:2 * jp + 2, :].rearrange(
                    "j (t n) -> t j n", t=128)
                nc.sync.dma_start(
                    dst,
                    u_all[:, 1024 * jp:1024 * (jp + 1)].rearrange(
                        "p (j n) -> p j n", j=2))

    nc.compile()
    return nc


def _get_nc():
    if "nc" not in _CACHE:
        _CACHE["nc"] = _build_nc()
    return _CACHE["nc"]


def _host_prep(z, t, W1, b1, W2, b2):
    f32 = np.float32
    z = np.asarray(z, f32)
    t = np.asarray(t, f32)
    W1 = np.asarray(W1, f32)
    b1 = np.asarray(b1, f32)
    W2 = np.asarray(W2, f32)
    b2 = np.asarray(b2, f32)

    f16 = np.float16
    inp_aug = np.zeros((14, _BPAD), f16)
    inp_aug[0, :_B] = t.astype(f16)
    inp_aug[1:13, :_B] = z.T.astype(f16)
    inp_aug[13, :] = 1.0

    W1a = np.concatenate([W1, b1[None, :]], axis=0).astype(f16)   # [14, 100]

    # negated, zero-padded to 32 columns so each mm2 writes a full 32-row
    # PSUM block (gap rows become finite zeros instead of stale garbage)
    W2cn = np.zeros((100, 32), np.float16)
    W2cn[:, 0] = (-(W2[:, 6] + W2[:, 7] + W2[:, 8]) / f32(_MASS)).astype(
        np.float16)
    W2cn[:, 1] = -W2[:, 9].astype(np.float16)
    W2cn[:, 2] = -W2[:, 10].astype(np.float16)
    W2cn[:, 3] = -W2[:, 11].astype(np.float16)

    b2cn = np.array([-(b2[6] + b2[7] + b2[8]) / _MASS,
                     -b2[9], -b2[10], -b2[11]], f32)
    b2s = np.zeros((100, 1), f32)                                 # sparse bias
    for k in range(3):
        b2s[32 * k:32 * k + 4, 0] = b2cn

    return inp_aug, W1a, W2cn, b2s


def kernel(z, t, W1, b1, W2, b2):
    from concourse.bass_utils import run_bass_kernel_spmd

    inp_aug, W1a, W2cn, b2s = _host_prep(z, t, W1, b1, W2, b2)
    nc = _get_nc()

    in_maps = []
    for c in range(_NCORES):
        in_maps.append({
            "inp": np.ascontiguousarray(
                inp_aug[:, _BLOC * c:_BLOC * (c + 1)]),
            "w1a": W1a,
            "w2cn": W2cn,
            "b2s": b2s,
        })

    res = run_bass_kernel_spmd(nc, in_maps, core_ids=list(range(_NCORES)))
    ut = np.concatenate([res.results[c]["out"] for c in range(_NCORES)],
                        axis=1)                                   # [4, BPAD]
    return np.ascontiguousarray(ut[:, :_B].T)                     # [B, 4]


# revision 22
# speedup vs baseline: 1.0092x; 1.0092x over previous
"""Trainium2 Bass kernel for nn_CVXPolicy_Quadcopter.

Computes, for each of B=500000 samples:
    p = MLP(concat([t, z]));  c = [(p6+p7+p8)/m, p9, p10, p11]
    ustar = -c * exp(-0.5 * W(||c||^2))   (W = Lambert W, via Newton)

Strategy (pure data parallel over 8 cores, feature-major MLP):
  - host: inp_aug [14, B_pad] = [t; z^T; ones] (ones row folds b1 into W1).
    W2 columns are pre-combined and NEGATED (W2cn) so the device computes
    -c directly; ||c||^2 is sign-invariant and ustar = (-c) * exp(-w/2).
  - device per core (65536 samples = 128 tiles of 512):
      mm1: h_pre[100, 512] = W1a^T @ inp_tile  (PE, K=14)
      tanh: ACT over [100, 2048] (4 tiles / PSUM-bank group)
      mm2: c[4, 512] per tile, packed 3 tiles/PSUM bank at offsets {0,32,64}
      bias-copy: PSUM -> SBUF with per-partition bias (DVE), then per-tile
      DMA folds [4, 512] -> c_all[tile, 2048] (component-major planes)
      dense phase on [128, 512]: x = ||c||^2, Lambert W via 4 Newton iters
      (init = exponent-bit log1p approx; exp on ACT, rest on DVE),
      ustar planes = c_all * exp(-w/2), one DMA out to u_t [4, 65536]
  - host: gather per-core u_t, transpose to [B, 4].
"""

import numpy as np

_B = 500000
_NCORES = 8
_BLOC = 65536            # per-core padded batch: 128 tiles x 512
_BPAD = _BLOC * _NCORES
_NT = 512                # samples per tile
_NTILES = _BLOC // _NT   # 128
_MASS = 0.5
# w0 = C * (int_bits(1 + x) - B): tuned so that 4 Newton iters reach fp32 eps
_LOG_B = 1064866805.0
_LOG_C = 6.197218803882235e-08
_NEWTON_ITERS = 4

_CACHE = {}


def _build_nc():
    import concourse.bacc as bacc
    import concourse.tile as tile
    from concourse import mybir

    f32 = mybir.dt.float32
    f16 = mybir.dt.float16
    i32 = mybir.dt.int32
    AF = mybir.ActivationFunctionType
    ALU = mybir.AluOpType

    nc = bacc.Bacc("TRN2", target_bir_lowering=False, debug=False,
                   num_devices=_NCORES)

    inp = nc.dram_tensor("inp", [14, _BLOC], f16, kind="ExternalInput")
    w1a = nc.dram_tensor("w1a", [14, 100], f16, kind="ExternalInput")
    w2cn = nc.dram_tensor("w2cn", [100, 32], f16, kind="ExternalInput")
    b2s = nc.dram_tensor("b2s", [100, 1], f32, kind="ExternalInput")
    out = nc.dram_tensor("out", [4, _BLOC], f32, kind="ExternalOutput")

    with tile.TileContext(nc) as tc:
        with (
            tc.tile_pool(name="consts", bufs=1) as consts,
            tc.tile_pool(name="inpp", bufs=3) as inp_pool,
            tc.tile_pool(name="hs", bufs=3) as h_pool,
            tc.tile_pool(name="csb", bufs=3) as csb_pool,
            tc.tile_pool(name="big", bufs=1) as big_pool,
            tc.tile_pool(name="nt", bufs=2) as nt_pool,
            tc.tile_pool(name="hp", bufs=2, space="PSUM") as hp_pool,
            tc.tile_pool(name="cps", bufs=2, space="PSUM") as c_pool,
            tc.tile_pool(name="dram", bufs=1, space="DRAM") as dram_pool,
        ):
            w1a_sb = consts.tile([14, 100], f16, tag="w1a")
            nc.sync.dma_start(w1a_sb, w1a[:])
            w2c_sb = consts.tile([100, 32], f16, tag="w2c")
            nc.sync.dma_start(w2c_sb, w2cn[:])
            b2_sb = consts.tile([100, 1], f32, tag="b2s")
            nc.sync.dma_start(b2_sb, b2s[:])

            c_all = big_pool.tile([128, 2048], f32, tag="c_all")
            u_all = big_pool.tile([128, 2048], f32, tag="u_all")
            half = consts.tile([128, 1], f32, tag="half")
            nc.vector.memset(half, 0.5)
            # DRAM scratch for the c densify round-trip: triad g's biased
            # c block [96, 512] is dumped verbatim; the readback gathers the
            # real rows {32k+i} of every tile with DRAM-side addressing
            scratch = dram_pool.tile([43, 96, 512], f32, tag="scratch")

            # dense-phase tiles (written in row halves; A half mid-loop)
            sq = nt_pool.tile([128, 2048], f32, tag="sq", bufs=1)
            s2 = nt_pool.tile([128, 1024], f32, tag="s2", bufs=1)
            x = nt_pool.tile([128, _NT], f32, tag="x", bufs=1)
            xe = nt_pool.tile([128, _NT], f32, tag="xe", bufs=1)
            y = nt_pool.tile([128, _NT], f32, tag="y", bufs=1)
            fi = nt_pool.tile([128, _NT], f32, tag="fi", bufs=1)
            v = nt_pool.tile([128, _NT], f32, tag="v")

            def dense_half(r0, r1):
                nc.vector.tensor_mul(sq[r0:r1], c_all[r0:r1], c_all[r0:r1])
                nc.vector.tensor_add(s2[r0:r1], sq[r0:r1, 0:1024],
                                     sq[r0:r1, 1024:2048])
                nc.vector.tensor_add(x[r0:r1], s2[r0:r1, 0:512],
                                     s2[r0:r1, 512:1024])
                nc.vector.tensor_scalar_add(y[r0:r1], x[r0:r1], 1.0)
                nc.vector.tensor_scalar_mul(xe[r0:r1], x[r0:r1],
                                            float(np.e))
                nc.vector.tensor_copy(fi[r0:r1], y[r0:r1].bitcast(i32))
                # v0 = C*(bits - B) + 1 = bits*C - (B*C - 1)
                nc.vector.tensor_scalar(
                    v[r0:r1], fi[r0:r1], _LOG_C, _LOG_B * _LOG_C - 1.0,
                    op0=ALU.mult, op1=ALU.subtract)

            NG = (_NTILES + 2) // 3   # 43 triads (last has 2 tiles)
            it_ring = {}
            for g in range(NG):
                ntr = min(3, _NTILES - 3 * g)
                t0 = 3 * g
                # issue the input DMA for any octet starting in this triad
                for tt in range(t0, t0 + ntr):
                    if tt % 8 == 0:
                        o = tt // 8
                        it_new = inp_pool.tile([14, 4096], f16, tag="inp")
                        it_ring[o] = it_new
                        nc.sync.dma_start(
                            it_new, inp[:, 4096 * o:4096 * (o + 1)])
                hp_cur = hp_pool.tile([100, 512 * ntr], f32, tag="hp")
                for k in range(ntr):
                    tt = t0 + k
                    nc.tensor.matmul(
                        hp_cur[:, _NT * k:_NT * (k + 1)],
                        lhsT=w1a_sb[:],
                        rhs=it_ring[tt // 8][:, _NT * (tt % 8):
                                             _NT * (tt % 8 + 1)],
                        start=True, stop=True,
                    )
                h_cur = h_pool.tile([100, 512 * ntr], f16, tag="h")
                nc.scalar.activation(h_cur, hp_cur, AF.Tanh)
                cps_cur = c_pool.tile([128, _NT], f32, tag="c")
                for k in range(ntr):
                    nc.tensor.matmul(
                        cps_cur[32 * k:32 * k + 32, :],
                        lhsT=w2c_sb[:],
                        rhs=h_cur[:, _NT * k:_NT * (k + 1)],
                        start=True, stop=True,
                    )
                nr = 32 * ntr
                cc = csb_pool.tile([128, _NT], f32, tag="csb")
                nc.vector.tensor_scalar_add(
                    cc[0:nr, :], cps_cur[0:nr, :], b2_sb[0:nr, :])
                # dump triad (incl. garbage rows) verbatim to DRAM scratch
                # on the otherwise-idle software DGE
                nc.gpsimd.dma_start(scratch[g, 0:nr, :], cc[0:nr, :])
                if g == 21:
                    # tiles 0..63 are all dumped; gather them now so the
                    # readback overlaps the remaining loop iterations
                    srcA = scratch[:].rearrange(
                        "g (k r) n -> (g k) r n", k=3)[0:64, 0:4, :]
                    nc.sync.dma_start(
                        c_all[0:64, :].rearrange("p (i n) -> p i n", i=4),
                        srcA)
                    dense_half(0, 64)

            srcB = scratch[:].rearrange(
                "g (k r) n -> (g k) r n", k=3)[64:128, 0:4, :]
            nc.sync.dma_start(
                c_all[64:128, :].rearrange("p (i n) -> p i n", i=4), srcB)

            dense_half(64, 128)
            for _ in range(_NEWTON_ITERS):
                f = nt_pool.tile([128, _NT], f32, tag="f")
                nc.scalar.activation(f, v, AF.Exp, scale=-1.0)
                rv = nt_pool.tile([128, _NT], f32, tag="rv")
                nc.vector.reciprocal(rv, v)
                p = nt_pool.tile([128, _NT], f32, tag="p")
                nc.vector.scalar_tensor_tensor(
                    p, v, 1.0, v, op0=ALU.subtract, op1=ALU.mult)
                tt_ = nt_pool.tile([128, _NT], f32, tag="tt")
                nc.vector.tensor_mul(tt_, xe, f)
                num = nt_pool.tile([128, _NT], f32, tag="num")
                nc.vector.scalar_tensor_tensor(
                    num, p, 1.0, tt_, op0=ALU.add, op1=ALU.add)
                vn = nt_pool.tile([128, _NT], f32, tag="v")
                nc.vector.tensor_mul(vn, num, rv)
                v = vn

            # ne = exp(-w/2) = exp(-(v-1)/2) = exp(-0.5*v + 0.5)
            ne = nt_pool.tile([128, _NT], f32, tag="ne", bufs=1)
            nc.scalar.activation(ne, v, AF.Exp, scale=-0.5, bias=half[:])

            for j in range(4):
                nc.vector.tensor_mul(
                    u_all[:, _NT * j:_NT * (j + 1)],
                    c_all[:, _NT * j:_NT * (j + 1)],
                    ne,
                )
                dst = out[j:j + 1, :].rearrange("j (t n) -> t j n", t=128)
                nc.sync.dma_start(
                    dst,
                    u_all[:, _NT * j:_NT * (j + 1)].rearrange(
                        "p (j n) -> p j n", j=1))

    nc.compile()
    return nc


def _get_nc():
    if "nc" not in _CACHE:
        _CACHE["nc"] = _build_nc()
    return _CACHE["nc"]


def _host_prep(z, t, W1, b1, W2, b2):
    f32 = np.float32
    z = np.asarray(z, f32)
    t = np.asarray(t, f32)
    W1 = np.asarray(W1, f32)
    b1 = np.asarray(b1, f32)
    W2 = np.asarray(W2, f32)
    b2 = np.asarray(b2, f32)

    f16 = np.float16
    inp_aug = np.zeros((14, _BPAD), f16)
    inp_aug[0, :_B] = t.astype(f16)
    inp_aug[1:13, :_B] = z.T.astype(f16)
    inp_aug[13, :] = 1.0

    W1a = np.concatenate([W1, b1[None, :]], axis=0).astype(f16)   # [14, 100]

    # negated, zero-padded to 32 columns so each mm2 writes a full 32-row
    # PSUM block (gap rows become finite zeros instead of stale garbage)
    W2cn = np.zeros((100, 32), np.float16)
    W2cn[:, 0] = (-(W2[:, 6] + W2[:, 7] + W2[:, 8]) / f32(_MASS)).astype(
        np.float16)
    W2cn[:, 1] = -W2[:, 9].astype(np.float16)
    W2cn[:, 2] = -W2[:, 10].astype(np.float16)
    W2cn[:, 3] = -W2[:, 11].astype(np.float16)

    b2cn = np.array([-(b2[6] + b2[7] + b2[8]) / _MASS,
                     -b2[9], -b2[10], -b2[11]], f32)
    b2s = np.zeros((100, 1), f32)                                 # sparse bias
    for k in range(3):
        b2s[32 * k:32 * k + 4, 0] = b2cn

    return inp_aug, W1a, W2cn, b2s


def kernel(z, t, W1, b1, W2, b2):
    from concourse.bass_utils import run_bass_kernel_spmd

    inp_aug, W1a, W2cn, b2s = _host_prep(z, t, W1, b1, W2, b2)
    nc = _get_nc()

    in_maps = []
    for c in range(_NCORES):
        in_maps.append({
            "inp": np.ascontiguousarray(
                inp_aug[:, _BLOC * c:_BLOC * (c + 1)]),
            "w1a": W1a,
            "w2cn": W2cn,
            "b2s": b2s,
        })

    res = run_bass_kernel_spmd(nc, in_maps, core_ids=list(range(_NCORES)))
    ut = np.concatenate([res.results[c]["out"] for c in range(_NCORES)],
                        axis=1)                                   # [4, BPAD]
    return np.ascontiguousarray(ut[:, :_B].T)                     # [B, 4]
